# revision 20
# baseline (speedup 1.0000x reference)
"""Trainium2 Bass kernel for nn_DecoderWithAttention (show-attend-tell decoder).

Sharding: data-parallel over batch — 8 batches per core x 8 NeuronCores,
weights replicated, no collectives.

Algebraic restructure:
  * context_t = sum_p alpha[b,p] * enc[b,p,:] feeds the LSTM only through
    Wih_ctx, so EncW = enc @ Wih_ctx.T is precomputed once per core and each
    step contracts alpha directly against EncW (block-diagonal alpha as the
    stationary PE operand) — the per-step 2048x2048 context projection
    disappears.
  * The vocab projection (fc) of every step's h is batched into one matmul
    after the recurrence.
  * All LSTM sigmoids are computed from tanh (sigm(x) = .5 + .5 tanh(x/2)) so
    relu/exp/tanh live in ONE ScalarE table set (exp_and_others).

Numerics: f32r (TF32-class) matmuls on the recurrence/fc path; bf16 on the
enc / EncW / attention streams; fp32 elementwise.
"""
import os
import numpy as np
import ml_dtypes

import concourse.bass as bass
import concourse.mybir as mybir
import concourse.tile as tile
from concourse.bass_utils import run_bass_kernel_spmd
from concourse.masks import make_identity

F32 = mybir.dt.float32
F32R = mybir.dt.float32r
BF16 = mybir.dt.bfloat16
I32 = mybir.dt.int32
AF = mybir.ActivationFunctionType
OP = mybir.AluOpType
AX = mybir.AxisListType


# ---------------------------------------------------------------------------
# walrus on this toolchain accepts only ONE sync wait per instruction, but
# Tile emits multi-wait instructions (epilogue drain, multi-producer
# consumers). Post-pass: keep one wait on the instruction, hoist the rest
# onto same-engine NOPs inserted immediately before it.
_ws_ctr = [0]


def split_multiwaits(nc):
    for fn in nc.m.functions:
        for blk in fn.blocks:
            out = []
            changed = False
            for inst in list(blk.instructions):
                si = inst.sync_info
                if si is not None and si.on_wait and len(si.on_wait) > 1:
                    waits = list(si.on_wait)
                    for w in waits[:-1]:
                        _ws_ctr[0] += 1
                        out.append(mybir.InstNoOp(
                            name=f"WSPLIT-{_ws_ctr[0]}", ins=[], outs=[],
                            engine=inst.engine,
                            sync_info=mybir.SyncInfo(on_wait=[w], on_update=[])))
                    inst.sync_info = mybir.SyncInfo(
                        on_wait=[waits[-1]], on_update=list(si.on_update))
                    changed = True
                out.append(inst)
            if changed:
                try:
                    blk.instructions = out
                except Exception:
                    while len(blk.instructions):
                        blk.instructions.pop()
                    blk.instructions.extend(out)
    return nc


B, P, ENC, E, D, A, V, T = 64, 196, 2048, 512, 512, 512, 10000, 32
NCORES = 8
BS = B // NCORES
G = 4 * D
TS = int(os.environ.get("KSTEPS", T - 1))
PPAD = 256
KT = BS * PPAD // 128       # 16 K-tiles in padded (b, p) space


_prog_cache = {}


def build_program(ts):
    tb = ts * BS
    nc = bass.Bass("TRN2", target_bir_lowering=False, debug=False)
    dt_ = nc.dram_tensor

    enc_d = dt_("enc", [BS, ENC, P], BF16, kind="ExternalInput").ap()
    idx_d = dt_("idx", [tb, 1], I32, kind="ExternalInput").ap()
    emb_d = dt_("embt", [V, E], F32, kind="ExternalInput").ap()
    encattT_d = dt_("encattT", [ENC, A], BF16, kind="ExternalInput").ap()
    wihctxT_d = dt_("wihctxT", [ENC, G], BF16, kind="ExternalInput").ap()
    wihembT_d = dt_("wihembT", [E, G], F32R, kind="ExternalInput").ap()
    whhT_d = dt_("whhT", [D, G], F32R, kind="ExternalInput").ap()
    decattT_d = dt_("decattT", [D, A], F32R, kind="ExternalInput").ap()
    winitT_d = dt_("winitT", [ENC, 2 * D], F32R, kind="ExternalInput").ap()
    fcT_d = dt_("fcT", [D, V], F32R, kind="ExternalInput").ap()
    w_d = dt_("wvec", [A, 1], BF16, kind="ExternalInput").ap()
    cbias_d = dt_("cbias", [128, 4], BF16, kind="ExternalInput").ap()
    lstmb_d = dt_("lstmb", [1, G], F32R, kind="ExternalInput").ap()
    initb_d = dt_("initb", [1, 2 * D], F32R, kind="ExternalInput").ap()
    fcb_d = dt_("fcb", [1, V], F32R, kind="ExternalInput").ap()
    ones_d = dt_("ones", [1, 256], F32R, kind="ExternalInput").ap()
    ident8_d = dt_("ident8", [8, 8], F32R, kind="ExternalInput").ap()
    gembD = dt_("gembD", [tb, G], F32R).ap()
    preds_d = dt_("preds", [BS, T, V], F32, kind="ExternalOutput").ap()
    alphas_d = dt_("alphas", [BS, T, P], F32, kind="ExternalOutput").ap()

    BPCOLS = [(0, 512), (512, 512), (1024, 512), (1536, 32)]   # over BS*P=1568
    GC = [(0, 512), (512, 512), (1024, 512), (1536, 512)]      # over G=2048

    with tile.TileContext(nc) as tc:
        with tc.tile_pool(name="const", bufs=1) as const, \
             tc.tile_pool(name="big", bufs=1) as big:

            whhT_t = const.tile([128, 4, G], F32R, tag="whhT")
            nc.sync.dma_start(whhT_t[:], whhT_d.rearrange("(c p) g -> p c g", p=128))
            decattT_t = const.tile([128, 4, A], F32R, tag="decattT")
            nc.sync.dma_start(decattT_t[:], decattT_d.rearrange("(c p) a -> p c a", p=128))
            w_t = const.tile([128, 4, 1], BF16, tag="wvec")
            nc.sync.dma_start(w_t[:], w_d.rearrange("(c p) o -> p c o", p=128))
            cbias_t = const.tile([128, 4], BF16, tag="cbias")
            nc.sync.dma_start(cbias_t[:], cbias_d[:])
            ones_t = const.tile([1, 256], F32R, tag="ones")
            nc.sync.dma_start(ones_t[:], ones_d[:])
            ident8_t = const.tile([8, 8], F32R, tag="ident8")
            nc.sync.dma_start(ident8_t[:], ident8_d[:])
            identf_t = const.tile([128, 128], F32, tag="identf")
            make_identity(nc, identf_t[:])

            enc_proj_t = big.tile([128, 4, BS * P], BF16, tag="encproj")
            encw_t = big.tile([128, KT, G], BF16, tag="encw")
            nc.vector.memset(encw_t[:], 0.0)
            # alpha block-diag: K-tile k occupies flat cols 8k..8k+8 (128 cols
            # used, padded to 144). Within K-tile 2b (rows p 0..127 of batch b)
            # alpha[b] must land in COLUMN b (the out partition): flat 16b+b=17b.
            # K-tile 2b+1 (rows p 128..195): flat 17b+8.
            ablk_t = big.tile([128, 144], BF16, tag="ablk")
            nc.vector.memset(ablk_t[:], 0.0)
            H_t = big.tile([128, 4, tb], F32R, tag="Hbig")
            h0_t = big.tile([128, 4, 8], F32R, tag="h0")
            c_t = big.tile([128, 4, 8], F32, tag="cstate")
            meanT_t = big.tile([128, 16, 8], F32, tag="meanT")
            meanR_t = big.tile([128, 16, 8], F32R, tag="meanR")

            # ============ P0/P1: load enc to SBUF; enc_proj + mean ============
            with tc.tile_pool(name="encsb", bufs=1) as encpool:
                enc_sb = encpool.tile([128, 16, BS, P], BF16, tag="encsb")
                for k in range(16):
                    nc.sync.dma_start(
                        enc_sb[:, k, :, :],
                        enc_d[:, 128 * k:128 * (k + 1), :].rearrange("b e p -> e b p"))
                with tc.tile_pool(name="p1s", bufs=3) as p1s, \
                     tc.tile_pool(name="p1p", bufs=1, space="PSUM") as p1p:
                    for half in range(2):
                        pps = [p1p.tile([128, 512], F32, tag=f"pep{i}", name=f"pep{i}") for i in range(8)]
                        for k in range(16):
                            wb = p1s.tile([128, 256], BF16, tag="eattstream")
                            nc.sync.dma_start(
                                wb[:], encattT_d[128 * k:128 * (k + 1),
                                                 256 * half:256 * (half + 1)])
                            if half == 0:
                                nc.vector.reduce_sum(meanT_t[:, k, :], enc_sb[:, k, :, :],
                                                     axis=AX.X)
                            flat = enc_sb[:, k, :, :].rearrange("p b q -> p (b q)")
                            for a2 in range(2):
                                for ci, (c0, cn) in enumerate(BPCOLS):
                                    nc.tensor.matmul(
                                        pps[a2 * 4 + ci][:, 0:cn],
                                        wb[:, 128 * a2:128 * (a2 + 1)],
                                        flat[:, c0:c0 + cn],
                                        start=(k == 0), stop=(k == 15))
                        for a2 in range(2):
                            for ci, (c0, cn) in enumerate(BPCOLS):
                                dst = enc_proj_t[:, 2 * half + a2, c0:c0 + cn]
                                if (a2 + ci) % 2 == 0:
                                    nc.vector.tensor_copy(dst, pps[a2 * 4 + ci][:, 0:cn])
                                else:
                                    nc.scalar.copy(dst, pps[a2 * 4 + ci][:, 0:cn])
                    nc.vector.tensor_scalar_mul(meanR_t[:], meanT_t[:], 1.0 / P)

                # ============ P2: EncW (reads cached enc) ============
                MG = [list(range(0, 8)), list(range(8, 16))]
                with tc.tile_pool(name="p2s", bufs=3) as p2s, \
                     tc.tile_pool(name="p2p", bufs=1, space="PSUM") as p2p:
                    for gq in range(4):
                        for mg in range(2):
                            mts = MG[mg]
                            pws = [p2p.tile([128, 512], F32, tag=f"pw{i}", name=f"pw{i}") for i in range(8)]
                            for k in range(16):
                                wc = p2s.tile([128, 512], BF16, tag="wihstream")
                                nc.sync.dma_start(
                                    wc[:], wihctxT_d[128 * k:128 * (k + 1),
                                                     512 * gq:512 * (gq + 1)])
                                flat = enc_sb[:, k, :, :].rearrange("p b q -> p (b q)")
                                for mi, m in enumerate(mts):
                                    b = (m * 128) // PPAD
                                    p0 = (m * 128) % PPAD
                                    rows = 128 if p0 == 0 else P - 128
                                    lcol = b * P + p0
                                    nc.tensor.matmul(
                                        pws[mi][0:rows, :], flat[:, lcol:lcol + rows],
                                        wc[:], start=(k == 0), stop=(k == 15))
                            for mi, m in enumerate(mts):
                                rows = 128 if (m * 128) % PPAD == 0 else P - 128
                                dst = encw_t[0:rows, m, 512 * gq:512 * (gq + 1)]
                                if mi % 2 == 0:
                                    nc.vector.tensor_copy(dst, pws[mi][0:rows, :])
                                else:
                                    nc.scalar.copy(dst, pws[mi][0:rows, :])

            # ============ P3: embeddings -> Gemb (DRAM); h0/c0 ============
            with tc.tile_pool(name="p3s", bufs=2) as p3s:
                n2 = tb - 128 if tb > 128 else 0
                nrows = [128, n2] if n2 > 0 else [min(tb, 128)]
                with tc.tile_pool(name="p3p", bufs=1, space="PSUM") as p3p:
                    idx0 = p3s.tile([128, 1], I32, tag="idx0")
                    nc.sync.dma_start(idx0[0:nrows[0], :], idx_d[0:nrows[0], :])
                    embs = p3s.tile([128, 2, E], F32, tag="embs")
                    nc.gpsimd.indirect_dma_start(
                        out=embs[0:nrows[0], 0, :], out_offset=None, in_=emb_d[:],
                        in_offset=bass.IndirectOffsetOnAxis(ap=idx0[0:nrows[0], 0:1], axis=0))
                    if n2 > 0:
                        idx1 = p3s.tile([128, 1], I32, tag="idx1")
                        nc.sync.dma_start(idx1[0:n2, :], idx_d[128:tb, :])
                        nc.gpsimd.indirect_dma_start(
                            out=embs[0:n2, 1, :], out_offset=None, in_=emb_d[:],
                            in_offset=bass.IndirectOffsetOnAxis(ap=idx1[0:n2, 0:1], axis=0))
                    embsT = p3s.tile([128, 4, 256], F32R, tag="embsT")
                    for half, nrow in enumerate(nrows):
                        for kc in range(4):
                            ptr = p3p.tile([128, 128], F32, tag="ptr")
                            nc.tensor.transpose(
                                ptr[:, 0:nrow], embs[0:nrow, half, 128 * kc:128 * (kc + 1)],
                                identf_t[0:nrow, 0:nrow])
                            nc.vector.tensor_copy(
                                embsT[:, kc, 128 * half:128 * half + nrow], ptr[:, 0:nrow])
                    # Gemb = embs @ WihembT + lstm_bias ; stage to DRAM
                    for mh, nrow in enumerate(nrows):
                        pg3 = p3p.tile([128, G], F32, tag="pgemb")
                        for ci in range(4):
                            for k in range(4):
                                wi = p3s.tile([128, 512], F32R, tag="wihembstream")
                                nc.sync.dma_start(
                                    wi[:], wihembT_d[128 * k:128 * (k + 1),
                                                     512 * ci:512 * (ci + 1)])
                                nc.tensor.matmul(
                                    pg3[0:nrow, 512 * ci:512 * (ci + 1)],
                                    embsT[:, k, 128 * mh:128 * mh + nrow], wi[:],
                                    start=(k == 0), stop=False)
                            nc.tensor.matmul(
                                pg3[0:nrow, 512 * ci:512 * (ci + 1)],
                                ones_t[0:1, 128 * mh:128 * mh + nrow],
                                lstmb_t_ap(nc, const, lstmb_d)[0:1, 512 * ci:512 * (ci + 1)],
                                start=False, stop=True, skip_group_check=True)
                        gsb = p3s.tile([128, G], F32R, tag="gembsb")
                        nc.vector.tensor_copy(gsb[0:nrow, :], pg3[0:nrow, :])
                        nc.sync.dma_start(gembD[128 * mh:128 * mh + nrow, :], gsb[0:nrow, :])
                # h0c0
                with tc.tile_pool(name="p3q", bufs=1, space="PSUM") as p3q:
                    ph = p3q.tile([8, 2 * D], F32, tag="ph0")
                    for ci in range(2):
                        for k in range(16):
                            wi2 = p3s.tile([128, 512], F32R, tag="winitstream")
                            nc.sync.dma_start(
                                wi2[:], winitT_d[128 * k:128 * (k + 1),
                                                 512 * ci:512 * (ci + 1)])
                            nc.tensor.matmul(ph[:, 512 * ci:512 * (ci + 1)],
                                             meanR_t[:, k, :], wi2[:],
                                             start=(k == 0), stop=False)
                        nc.tensor.matmul(ph[:, 512 * ci:512 * (ci + 1)], ones_t[0:1, 0:8],
                                         initb_t_ap(nc, const, initb_d)[0:1, 512 * ci:512 * (ci + 1)],
                                         start=False, stop=True, skip_group_check=True)
                    hc_sb = p3s.tile([8, 2 * D], F32, tag="hcsb")
                    nc.vector.tensor_copy(hc_sb[:], ph[:])
                    for i in range(8):
                        ptp = p3q.tile([128, 8], F32, tag="ptp")
                        nc.tensor.transpose(ptp[:], hc_sb[0:8, 128 * i:128 * (i + 1)],
                                            identf_t[0:8, 0:8])
                        if i < 4:
                            nc.vector.tensor_copy(h0_t[:, i, :], ptp[:])
                        else:
                            nc.vector.tensor_copy(c_t[:, i - 4, :], ptp[:])

            # ============ P4: decode steps ============
            with tc.tile_pool(name="lw", bufs=2) as lw, \
                 tc.tile_pool(name="lw1", bufs=1) as lw1, \
                 tc.tile_pool(name="lps", bufs=1, space="PSUM") as lps, \
                 tc.tile_pool(name="lps2", bufs=2, space="PSUM") as lps2:
                for t in range(ts):
                    def hsl(c, _t=t):
                        if _t == 0:
                            return h0_t[:, c, :]
                        return H_t[:, c, (_t - 1) * 8:_t * 8]
                    # ---- dec_proj ----
                    pd = lps.tile([8, A], F32, tag="pgates", name="pd")
                    for k in range(4):
                        nc.tensor.matmul(pd[:], hsl(k), decattT_t[:, k, :],
                                         start=(k == 0), stop=(k == 3))
                    pd_sb = lw.tile([8, A], F32, tag="pdsb")
                    nc.vector.tensor_copy(pd_sb[:], pd[:])
                    ptd = lps2.tile([128, 4, 8], F32, tag="psmall")
                    for i in range(4):
                        nc.tensor.transpose(ptd[:, i, :],
                                            pd_sb[0:8, 128 * i:128 * (i + 1)],
                                            identf_t[0:8, 0:8])
                    bf_t = lw.tile([128, 4, 8], BF16, tag="bft")
                    nc.vector.tensor_tensor(
                        bf_t[:], ptd[:],
                        cbias_t[:].unsqueeze(-1).broadcast_to((128, 4, 8)), OP.add)
                    # ---- gates: Gemb inject + h@WhhT first (PE fills while ACT/DVE do att) ----
                    gbuf = lw.tile([8, G], F32R, tag="gembstep")
                    nc.sync.dma_start(gbuf[:], gembD[t * 8:(t + 1) * 8, :])
                    pg = lps.tile([8, G], F32, tag="pgates", name="pg")
                    for ci, (c0, cn) in enumerate(GC):
                        nc.tensor.matmul(pg[:, c0:c0 + cn], ident8_t[:],
                                         gbuf[:, c0:c0 + cn], start=True, stop=False)
                        for k in range(4):
                            nc.tensor.matmul(pg[:, c0:c0 + cn], hsl(k),
                                             whhT_t[:, k, c0:c0 + cn],
                                             start=False, stop=False,
                                             skip_group_check=True)
                    # ---- att/relu/e ----
                    ratt = lw1.tile([128, 4, BS * P], BF16, tag="ratt")
                    for c in range(4):
                        slot = c
                        if c < 2:
                            for b in range(BS):
                                nc.scalar.activation(
                                    ratt[:, slot, P * b:P * (b + 1)],
                                    enc_proj_t[:, c, P * b:P * (b + 1)], AF.Relu,
                                    bias=bf_t[:, c, b:b + 1])
                        else:
                            nc.vector.tensor_tensor(
                                ratt[:, slot, :].rearrange("p (b q) -> p b q", b=8),
                                enc_proj_t[:, c, :].rearrange("p (b q) -> p b q", b=8),
                                bf_t[:, c, :].unsqueeze(-1).broadcast_to((128, 8, P)),
                                OP.add)
                            nc.vector.tensor_scalar_max(
                                ratt[:, slot, :], ratt[:, slot, :], 0.0)
                    pes = []
                    for ci, (c0, cn) in enumerate(BPCOLS):
                        pec = lps2.tile([1, 512], F32, tag="pe", name="pec")
                        for c in range(4):
                            nc.tensor.matmul(pec[:, 0:cn], w_t[:, c, :],
                                             ratt[:, c, c0:c0 + cn],
                                             start=(c == 0), stop=(c == 3))
                        pes.append((pec, c0, cn))
                    e_row = lw.tile([1, BS * P], F32, tag="erow")
                    for pec, c0, cn in pes:
                        nc.vector.tensor_copy(e_row[:, c0:c0 + cn], pec[:, 0:cn])
                    e8 = lw.tile([8, P], F32, tag="e8")
                    nc.sync.dma_start(e8[:], e_row[:])
                    # ---- softmax ----
                    mx = lw.tile([8, 1], F32, tag="mx")
                    nc.vector.tensor_reduce(mx[:], e8[:], axis=AX.X, op=OP.max,
                                            negate=True)
                    ex = lw.tile([8, P], F32, tag="ex")
                    sm = lw.tile([8, 1], F32, tag="sm")
                    nc.scalar.activation(ex[:], e8[:], AF.Exp, bias=mx[:],
                                         accum_out=sm[:])
                    rc = lw.tile([8, 1], F32, tag="rc")
                    nc.vector.reciprocal(rc[:], sm[:])
                    alpha = lw.tile([8, P], F32, tag="alpha")
                    nc.vector.tensor_scalar_mul(alpha[:], ex[:], rc[:])
                    nc.sync.dma_start(alphas_d[:, t, :], alpha[:])
                    # ---- alpha -> block-diag via PE transpose ----
                    pat = lps2.tile([128, 2, 8], F32, tag="psmall")
                    nc.tensor.transpose(pat[:, 0, :], alpha[0:8, 0:128],
                                        identf_t[0:8, 0:8])
                    nc.tensor.transpose(pat[0:P - 128, 1, :], alpha[0:8, 128:P],
                                        identf_t[0:8, 0:8])
                    ablk_v = ablk_t[:, 0:136].rearrange("p (b r) -> p b r", r=17)
                    nc.vector.tensor_copy(ablk_v[:, :, 0], pat[:, 0, :])
                    nc.vector.tensor_copy(ablk_v[0:P - 128, :, 8],
                                          pat[0:P - 128, 1, :])
                    # ---- gates: alpha-blk @ EncW ----
                    for ci, (c0, cn) in enumerate(GC):
                        for k in range(KT):
                            koff = 8 * k
                            rows = 128 if k % 2 == 0 else P - 128
                            nc.tensor.matmul(pg[:, c0:c0 + cn],
                                             ablk_t[0:rows, koff:koff + 8],
                                             encw_t[0:rows, k, c0:c0 + cn],
                                             start=False, stop=(k == KT - 1),
                                             skip_group_check=True)
                    g_sb = lw.tile([8, G], F32, tag="gsb")
                    nc.vector.tensor_copy(g_sb[:, 0:1024], pg[:, 0:1024])
                    nc.scalar.copy(g_sb[:, 1024:2048], pg[:, 1024:2048])
                    pgt = lps2.tile([128, 16, 8], F32, tag="psmall")
                    for i in range(16):
                        nc.tensor.transpose(pgt[:, i, :],
                                            g_sb[0:8, 128 * i:128 * (i + 1)],
                                            identf_t[0:8, 0:8])
                    T_t = lw.tile([128, 16, 8], F32, tag="Ttanh")
                    nc.scalar.activation(T_t[:, 0:8, :], pgt[:, 0:8, :], AF.Tanh,
                                         scale=0.5)
                    nc.scalar.activation(T_t[:, 8:12, :], pgt[:, 8:12, :], AF.Tanh)
                    nc.scalar.activation(T_t[:, 12:16, :], pgt[:, 12:16, :], AF.Tanh,
                                         scale=0.5)
                    Ti, Tf = T_t[:, 0:4, :], T_t[:, 4:8, :]
                    Tg, To = T_t[:, 8:12, :], T_t[:, 12:16, :]
                    u = lw.tile([128, 4, 8], F32, tag="u")
                    nc.vector.tensor_tensor(u[:], Tf, c_t[:], OP.mult)
                    v = lw.tile([128, 4, 8], F32, tag="v")
                    nc.vector.tensor_tensor(v[:], Ti, Tg, OP.mult)
                    s1 = lw.tile([128, 4, 8], F32, tag="s1")
                    nc.vector.tensor_tensor(s1[:], c_t[:], Tg, OP.add)
                    nc.vector.tensor_tensor(u[:], u[:], v[:], OP.add)
                    nc.vector.tensor_tensor(s1[:], s1[:], u[:], OP.add)
                    nc.vector.tensor_scalar_mul(c_t[:], s1[:], 0.5)
                    tc2 = lw.tile([128, 4, 8], F32, tag="tc2")
                    nc.scalar.activation(tc2[:], c_t[:], AF.Tanh)
                    nc.vector.tensor_tensor(v[:], To, tc2[:], OP.mult)
                    nc.vector.tensor_tensor(v[:], v[:], tc2[:], OP.add)
                    nc.vector.tensor_scalar_mul(
                        H_t[:, :, t * 8:(t + 1) * 8], v[:], 0.5)

            # ============ P5: fc ============
            with tc.tile_pool(name="p5s", bufs=2) as p5s, \
                 tc.tile_pool(name="p5p", bufs=2, space="PSUM") as p5p:
                nvc = (V + 511) // 512
                for vi in range(nvc):
                    v0 = vi * 512
                    vn = min(512, V - v0)
                    fbufs = []
                    for k in range(4):
                        fb = p5s.tile([128, 512], F32R, tag=f"fcs{k}", name=f"fcs{k}")
                        nc.sync.dma_start(fb[:, 0:vn],
                                          fcT_d[128 * k:128 * (k + 1), v0:v0 + vn])
                        fbufs.append(fb)
                    fbb = p5s.tile([1, 512], F32R, tag="fcbs")
                    nc.sync.dma_start(fbb[0:1, 0:vn], fcb_d[0:1, v0:v0 + vn])
                    for mh in range(2 if tb > 128 else 1):
                        nrow = min(128, tb) if mh == 0 else tb - 128
                        t0 = (128 * mh) // 8
                        nt = nrow // 8
                        pf = p5p.tile([128, 512], F32, tag="pf")
                        for k in range(4):
                            nc.tensor.matmul(pf[0:nrow, 0:vn],
                                             H_t[:, k, 128 * mh:128 * mh + nrow],
                                             fbufs[k][:, 0:vn],
                                             start=(k == 0), stop=False)
                        nc.tensor.matmul(pf[0:nrow, 0:vn],
                                         ones_t[0:1, 0:nrow], fbb[0:1, 0:vn],
                                         start=False, stop=True,
                                         skip_group_check=True)
                        osb = p5s.tile([128, 512], F32, tag="osb")
                        if vi % 2 == 0:
                            nc.vector.tensor_copy(osb[0:nrow, 0:vn], pf[0:nrow, 0:vn])
                        else:
                            nc.scalar.copy(osb[0:nrow, 0:vn], pf[0:nrow, 0:vn])
                        nc.sync.dma_start(
                            preds_d[:, t0:t0 + nt, v0:v0 + vn].transpose((1, 0, 2)),
                            osb[0:nrow, 0:vn])

    split_multiwaits(nc)
    return nc


_side = {}


def lstmb_t_ap(nc, const, lstmb_d):
    if "lstmb" not in _side:
        t = const.tile([1, G], F32R, tag="lstmb")
        nc.sync.dma_start(t[:], lstmb_d[:])
        _side["lstmb"] = t
    return _side["lstmb"][:]


def initb_t_ap(nc, const, initb_d):
    if "initb" not in _side:
        t = const.tile([1, 2 * D], F32R, tag="initb")
        nc.sync.dma_start(t[:], initb_d[:])
        _side["initb"] = t
    return _side["initb"][:]


def kernel(encoder_out, captions, lengths, emb_table,
           enc_att_W, enc_att_b, dec_att_W, dec_att_b, full_att_W, full_att_b,
           inith_W, inith_b, initc_W, initc_b,
           lstm_Wih, lstm_bih, lstm_bhh, lstm_Whh, fc_W, fc_b):
    f32 = lambda x: np.ascontiguousarray(np.asarray(x, dtype=np.float32))
    bf16 = lambda x: np.ascontiguousarray(
        np.asarray(x, dtype=np.float32).astype(ml_dtypes.bfloat16))
    encoder_out = f32(encoder_out)
    captions = np.asarray(captions).astype(np.int32)
    ts = TS
    Wih = f32(lstm_Wih)
    shared = {
        "embt": f32(emb_table),
        "encattT": bf16(f32(enc_att_W).T),
        "wihctxT": bf16(Wih[:, E:].T),
        "wihembT": np.ascontiguousarray(Wih[:, 0:E].T),
        "whhT": np.ascontiguousarray(f32(lstm_Whh).T),
        "decattT": np.ascontiguousarray(f32(dec_att_W).T),
        "winitT": np.ascontiguousarray(
            np.concatenate([f32(inith_W), f32(initc_W)], axis=0).T),
        "fcT": np.ascontiguousarray(f32(fc_W).T),
        "wvec": bf16(f32(full_att_W).reshape(A, 1)),
        "cbias": bf16((f32(enc_att_b) + f32(dec_att_b)).reshape(4, 128).T),
        "lstmb": (f32(lstm_bih) + f32(lstm_bhh)).reshape(1, G),
        "initb": np.concatenate([f32(inith_b), f32(initc_b)]).reshape(1, 2 * D),
        "fcb": f32(fc_b).reshape(1, V),
        "ones": np.ones((1, 256), np.float32),
        "ident8": np.eye(8, dtype=np.float32),
    }
    global _side
    if ts not in _prog_cache:
        _side = {}
        _prog_cache[ts] = build_program(ts)
    nc = _prog_cache[ts]

    in_maps = []
    for c in range(NCORES):
        bs = slice(c * BS, (c + 1) * BS)
        m = dict(shared)
        m["enc"] = bf16(encoder_out[bs])
        m["idx"] = np.ascontiguousarray(
            captions[bs, 0:ts].T.reshape(ts * BS, 1).astype(np.int32))
        in_maps.append(m)

    res = run_bass_kernel_spmd(nc, in_maps, list(range(NCORES)))
    preds = np.zeros((B, T, V), np.float32)
    alphas = np.zeros((B, T, P), np.float32)
    for c in range(NCORES):
        preds[c * BS:(c + 1) * BS] = res.results[c]["preds"]
        alphas[c * BS:(c + 1) * BS] = res.results[c]["alphas"]
    return preds, alphas
